# revision 2
# baseline (speedup 1.0000x reference)
"""JointAngleLoss Trainium2 kernel (8-core data-parallel).

Input : pose23d_pred [524288, 21, 3] float32
Output: scalar float32 loss (matches reference.reference)

Strategy: pure data-parallel over the batch dim. Each of the 8 NeuronCores
processes 65536 rows. Host pre-permutes the input (dtype-preserving) into a
per-(group, partition) component-planar layout so every device-side read is
contiguous. On device:
  DMA (fp32) -> ACT cast fp32->fp16 -> DVE bones/crosses/dot products (fp16,
  2x perf mode) -> ACT relu(-v)^2 with fp32 accum  +  PE ones-matmul reduction
  of the coplanarity products into PSUM (fp32)
Host sums the per-core partial sums in float64.
"""

import sys

for _p in ("/opt/trn_rl_repo", "/root/.axon_site/_ro/trn_rl_repo"):
    if _p not in sys.path:
        sys.path.append(_p)

import numpy as np

import concourse.bacc as bacc
import concourse.mybir as mybir
from concourse import tile
from concourse.bass_utils import run_bass_kernel_spmd
from contextlib import ExitStack

N_CORES = 8
P = 128          # SBUF partitions
B_FULL = 524288  # total batch
NJ = 21          # joints
NC3 = 3          # xyz
ROW = NJ * NC3   # 63 floats per row

F16 = mybir.dt.float16
F32 = mybir.dt.float32


def build_bass(rows_per_core: int, K: int, reps: int = 1):
    """Build the per-core Bass program.

    rows_per_core = P * K * G.  K = batch rows per partition slot per group.
    reps > 1 repeats the whole compute loop (timing variants only).
    Returns (nc, G).
    """
    assert rows_per_core % (P * K) == 0
    G = rows_per_core // (P * K)
    FK = ROW * K          # fp32 elems per partition per group (63*K)
    NB = 20 * K           # bone elems per component per partition
    NCOP = 15 * K         # coplane products per partition (3c * 5f * K)
    NV = 10 * K           # v values per partition (2 * 5f * K)

    nc = bacc.Bacc("TRN2", target_bir_lowering=False, debug=False)

    x = nc.dram_tensor("x", [G, P, FK], F32, kind="ExternalInput")
    cop_out = nc.dram_tensor("cop_out", [1, NCOP], F32, kind="ExternalOutput")
    mask_out = nc.dram_tensor("mask_out", [P, G * reps], F32, kind="ExternalOutput")

    with tile.TileContext(nc) as tc, ExitStack() as ctx:
        xpool = ctx.enter_context(tc.tile_pool(name="xpool", bufs=2))
        hpool = ctx.enter_context(tc.tile_pool(name="hpool", bufs=2))
        bpool = ctx.enter_context(tc.tile_pool(name="bpool", bufs=2))
        rpool = ctx.enter_context(tc.tile_pool(name="rpool", bufs=2))
        tpool = ctx.enter_context(tc.tile_pool(name="tpool", bufs=1))
        vpool = ctx.enter_context(tc.tile_pool(name="vpool", bufs=2))
        spool = ctx.enter_context(tc.tile_pool(name="spool", bufs=1))
        psum = ctx.enter_context(tc.tile_pool(name="psum", bufs=1, space="PSUM"))

        ones = spool.tile([P, 1], F16)
        nc.gpsimd.memset(ones[:], 1.0)
        acc = spool.tile([P, G * reps], F32)
        psum_cop = psum.tile([1, NCOP], F32)

        n_chunks = (NCOP + 511) // 512

        for rep in range(reps):
            for g in range(G):
                first = rep == 0 and g == 0
                last = rep == reps - 1 and g == G - 1

                # ---- load + cast -------------------------------------------------
                xt = xpool.tile([P, FK], F32)
                nc.sync.dma_start(xt[:], x.ap()[g])
                xh = hpool.tile([P, FK], F16)
                nc.scalar.copy(xh[:], xt[:])

                # xh layout per partition: [c:3][j:21][k:K]
                xhv = xh[:].rearrange("p (c j k) -> p c j k", c=3, j=NJ)

                # ---- bones: B[c][j][k] = X[c][j+1][k] - X[c][j][k], j=0..19 ------
                bones = bpool.tile([P, 3 * NB], F16)
                bv = bones[:].rearrange("p (c j k) -> p c j k", c=3, j=20)
                nc.vector.tensor_sub(bv, xhv[:, :, 1:, :], xhv[:, :, :20, :])
                # per-component view [jj:4][f:5][k]  (bone j = 4f + jj)
                bf = [
                    bones[:].rearrange("p (c f jj k) -> p c jj f k", c=3, f=5, jj=4)[:, c]
                    for c in range(3)
                ]

                # ---- crosses: rot_q = b_{q+1} x b_q, q=1..3 ----------------------
                # R_c[qh][f][k], qh=q-1: 0=palm(b2xb1) 1=mid(b3xb2) 2=tip(b4xb3)
                # rot_q[c] = B_{c1}[jj=qh+1]*B_{c2}[jj=qh] - B_{c2}[jj=qh+1]*B_{c1}[jj=qh]
                rot = []
                for c in range(3):
                    c1, c2 = (c + 1) % 3, (c + 2) % 3
                    m1 = tpool.tile([P, NCOP], F16, tag="m1")
                    m2 = tpool.tile([P, NCOP], F16, tag="m2")
                    rc = rpool.tile([P, NCOP], F16, tag=f"rot{c}")
                    m1v = m1[:].rearrange("p (q f k) -> p q f k", q=3, f=5)
                    m2v = m2[:].rearrange("p (q f k) -> p q f k", q=3, f=5)
                    nc.vector.tensor_mul(m1v, bf[c1][:, 1:4], bf[c2][:, 0:3])
                    nc.vector.tensor_mul(m2v, bf[c2][:, 1:4], bf[c1][:, 0:3])
                    rcv = rc[:].rearrange("p (q f k) -> p q f k", q=3, f=5)
                    nc.vector.tensor_sub(rcv, m1v, m2v)
                    rot.append(rc)

                # ---- coplane products: (palm_c + mid_c) * b4_c -------------------
                red = vpool.tile([P, NCOP], F16, tag="red")
                redv = red[:].rearrange("p (c f k) -> p c f k", c=3, f=5)
                for c in range(3):
                    rcv = rot[c][:].rearrange("p (q f k) -> p q f k", q=3, f=5)
                    pc = tpool.tile([P, 5 * K], F16, tag="pc")
                    pcv = pc[:].rearrange("p (f k) -> p f k", f=5)
                    nc.vector.tensor_add(pcv, rcv[:, 0], rcv[:, 1])
                    nc.vector.tensor_mul(redv[:, c], pcv, bf[c][:, 3])

                # ---- v1 = tip.mid, v2 = palm.mid  -> v[i:2][f][k] ----------------
                pprod = []
                for c in range(3):
                    rcv = rot[c][:].rearrange("p (q f k) -> p q f k", q=3, f=5)
                    pp = tpool.tile([P, NV], F16, tag=f"pp{c}")
                    ppv = pp[:].rearrange("p (i f k) -> p i f k", i=2, f=5)
                    nc.vector.tensor_mul(
                        ppv, rcv[:, 2::-2], rcv[:, 1:2].broadcast_to((P, 2, 5, K))
                    )
                    pprod.append(pp)
                vsum = tpool.tile([P, NV], F16, tag="vsum")
                nc.vector.tensor_add(vsum[:], pprod[0][:], pprod[1][:])
                v = vpool.tile([P, NV], F16, tag="v")
                nc.vector.tensor_add(v[:], vsum[:], pprod[2][:])

                # ---- masked squares on ACT: sum(relu(-v)^2) -> acc[:, idx] -------
                mrelu = vpool.tile([P, NV], F16, tag="mrelu")
                nc.scalar.activation(mrelu[:], v[:], mybir.ActivationFunctionType.Relu,
                                     scale=-1.0)
                sqj = vpool.tile([P, NV], F16, tag="sqj")
                nc.scalar.activation(sqj[:], mrelu[:], mybir.ActivationFunctionType.Square,
                                     accum_out=acc[:, rep * G + g : rep * G + g + 1])

                # ---- PE reduction of coplane products over partitions ------------
                for i in range(n_chunks):
                    lo = 512 * i
                    hi = min(NCOP, lo + 512)
                    nc.tensor.matmul(
                        psum_cop[:, lo:hi],
                        ones[:],
                        red[:, lo:hi],
                        start=first,
                        stop=last,
                    )

        # ---- epilogue: PSUM -> SBUF -> DRAM ---------------------------------
        cop_sb = spool.tile([1, NCOP], F32)
        nc.scalar.copy(cop_sb[:], psum_cop[:])
        nc.sync.dma_start(cop_out.ap(), cop_sb[:])
        nc.sync.dma_start(mask_out.ap(), acc[:])

    nc.compile()
    return nc, G


def host_planarize(x: np.ndarray, n_cores: int, K: int) -> np.ndarray:
    """[B, 21, 3] f32 -> [n_cores, G, P, 63*K] f32 planar per partition-slot.

    Per (core, group, partition): slot holds K batch rows laid out [c][j][k].
    """
    B = x.shape[0]
    R = B // n_cores
    G = R // (P * K)
    xr = x.reshape(n_cores, G, P, K, NJ, NC3)
    xp = xr.transpose(0, 1, 2, 5, 4, 3)  # -> [cores, G, P, 3, 21, K]
    return np.ascontiguousarray(xp).reshape(n_cores, G, P, ROW * K)


_CACHE = {}


def _get_nc(rows_per_core: int, K: int):
    key = (rows_per_core, K)
    if key not in _CACHE:
        _CACHE[key] = build_bass(rows_per_core, K)
    return _CACHE[key]


def kernel(pose23d_pred: np.ndarray) -> np.ndarray:
    x = np.asarray(pose23d_pred, dtype=np.float32)
    assert x.shape == (B_FULL, NJ, NC3), x.shape
    K = 128
    R = B_FULL // N_CORES
    nc, G = _get_nc(R, K)
    xp = host_planarize(x, N_CORES, K)
    in_maps = [{"x": xp[i]} for i in range(N_CORES)]
    res = run_bass_kernel_spmd(nc, in_maps, list(range(N_CORES)))
    total = 0.0
    for r in res.results:
        total += r["cop_out"].astype(np.float64).sum()
        total += r["mask_out"].astype(np.float64).sum()
    return np.float32(total)


# revision 8
# speedup vs baseline: 763.0874x; 763.0874x over previous
"""JointAngleLoss Trainium2 kernel (8-core data-parallel).

Input : pose23d_pred [524288, 21, 3] float32
Output: scalar float32 loss (matches reference.reference)

Strategy: pure data-parallel over the batch dim; each of 8 NeuronCores handles
65536 rows. Host pre-permutes the input (dtype preserving) into a per-partition
slot layout J[c][jj][f][k] (jj = joint-within-finger, duplicating the 4 shared
joints: 75 floats per row) so that EVERY device-side vector operand is a flat
contiguous fp16 slice - this keeps the DVE in its 2x_1P packed perf mode.
Device pipeline per group:
  DMA fp32 -> ACT cast->fp16 -> DVE bones/crosses/dots (flat fp16 2x)
  -> ACT relu(-v)^2 with fp32 accum_out  +  PE ones-matmul reduces the
  coplanarity products into PSUM fp32 across groups.
Host sums the per-core partials in float64.
"""

import sys

for _p in ("/opt/trn_rl_repo", "/root/.axon_site/_ro/trn_rl_repo"):
    if _p not in sys.path:
        sys.path.append(_p)

import numpy as np

import concourse.bacc as bacc
import concourse.mybir as mybir
from concourse import tile
from concourse.bass_utils import run_bass_kernel_spmd
from contextlib import ExitStack

N_CORES = 8
P = 128          # SBUF partitions
B_FULL = 524288  # total batch
ROW = 75         # 3 comps * 5 joint-slots * 5 fingers (shared joints duplicated)

F16 = mybir.dt.float16
F32 = mybir.dt.float32


def build_bass(rows_per_core: int, K: int, reps: int = 1, hw_loop: int = 1):
    """rows_per_core = P * K * G.  K = rows per partition slot per group.

    reps>1 unrolls the compute (timing); hw_loop>1 wraps it in a device-side
    For_i (timing; outputs = last iteration's = one correct pass).
    """
    assert rows_per_core % (P * K) == 0
    G = rows_per_core // (P * K)
    FK = ROW * K          # fp32 elems per partition per group (75*K)
    CJ = 25 * K           # joint elems per component (5jj*5f*K)
    CB = 20 * K           # bone elems per component  (4jj*5f*K)
    S5 = 5 * K            # one [f][k] slab
    NCOP = 3 * S5         # coplane products per partition
    NV = 2 * S5           # v values per partition

    nc = bacc.Bacc("TRN2", target_bir_lowering=False, debug=False)

    x = nc.dram_tensor("x", [G, P, FK], F32, kind="ExternalInput")
    cop_out = nc.dram_tensor("cop_out", [1, NCOP], F32, kind="ExternalOutput")
    mask_out = nc.dram_tensor("mask_out", [P, G * reps], F32, kind="ExternalOutput")

    with tile.TileContext(nc) as tc, ExitStack() as ctx:
        xpool = ctx.enter_context(tc.tile_pool(name="xpool", bufs=2))
        hpool = ctx.enter_context(tc.tile_pool(name="hpool", bufs=1))
        bpool = ctx.enter_context(tc.tile_pool(name="bpool", bufs=2))
        rpool = ctx.enter_context(tc.tile_pool(name="rpool", bufs=2))
        tpool = ctx.enter_context(tc.tile_pool(name="tpool", bufs=1))
        vpool = ctx.enter_context(tc.tile_pool(name="vpool", bufs=2))
        spool = ctx.enter_context(tc.tile_pool(name="spool", bufs=1))
        psum = ctx.enter_context(tc.tile_pool(name="psum", bufs=1, space="PSUM"))

        ones = spool.tile([P, 1], F16)
        nc.gpsimd.memset(ones[:], 1.0)
        acc = spool.tile([P, G * reps], F32)
        psum_cop = psum.tile([1, NCOP], F32)

        n_chunks = (NCOP + 511) // 512

        loop_cm = tc.For_i(0, hw_loop, 1) if hw_loop > 1 else None
        if loop_cm is not None:
            loop_cm.__enter__()

        for rep in range(reps):
            for g in range(G):
                first = rep == 0 and g == 0
                last = rep == reps - 1 and g == G - 1

                # ---- load + cast (all flat) ---------------------------------
                xt = xpool.tile([P, FK], F32)
                nc.sync.dma_start(xt[:], x.ap()[g])
                xh = hpool.tile([P, FK], F16)
                nc.scalar.copy(xh[:], xt[:])

                # ---- bones: B[c][jj][f][k] = J[c][jj+1][f][k]-J[c][jj][f][k]
                bones = bpool.tile([P, 3 * CB], F16)
                for c in range(3):
                    nc.vector.tensor_sub(
                        bones[:, c * CB : (c + 1) * CB],
                        xh[:, c * CJ + S5 : c * CJ + CJ],
                        xh[:, c * CJ : c * CJ + CB],
                    )

                def bslab(c, jj):  # bone block, flat [P, 5K]
                    o = c * CB + jj * S5
                    return bones[:, o : o + S5]

                # ---- crosses: R_c[qh][f][k], qh: 0=palm 1=mid 2=tip ---------
                # rot[c] = B_{c1}[jj=qh+1]*B_{c2}[jj=qh] - B_{c2}[jj=qh+1]*B_{c1}[jj=qh]
                rot = []
                for c in range(3):
                    c1, c2 = (c + 1) % 3, (c + 2) % 3
                    m1 = tpool.tile([P, NCOP], F16, tag="m1")
                    m2 = tpool.tile([P, NCOP], F16, tag="m2")
                    rc = rpool.tile([P, NCOP], F16, tag=f"rot{c}")
                    nc.vector.tensor_mul(
                        m1[:], bones[:, c1 * CB + S5 : c1 * CB + CB],
                        bones[:, c2 * CB : c2 * CB + NCOP])
                    nc.vector.tensor_mul(
                        m2[:], bones[:, c2 * CB + S5 : c2 * CB + CB],
                        bones[:, c1 * CB : c1 * CB + NCOP])
                    nc.vector.tensor_sub(rc[:], m1[:], m2[:])
                    rot.append(rc)

                # ---- coplane products: (palm_c + mid_c) * b4_c  (all flat) --
                red = vpool.tile([P, NCOP], F16, tag="red")
                for c in range(3):
                    pc = tpool.tile([P, S5], F16, tag="pc")
                    nc.vector.tensor_add(pc[:], rot[c][:, 0:S5], rot[c][:, S5:2 * S5])
                    nc.vector.tensor_mul(
                        red[:, c * S5 : (c + 1) * S5], pc[:], bslab(c, 3))

                # ---- v1 = tip.mid, v2 = palm.mid ----------------------------
                pprod = []
                for c in range(3):
                    pp = tpool.tile([P, NV], F16, tag=f"pp{c}")
                    nc.vector.tensor_mul(
                        pp[:, 0:S5], rot[c][:, 2 * S5 : 3 * S5], rot[c][:, S5 : 2 * S5])
                    nc.vector.tensor_mul(
                        pp[:, S5:NV], rot[c][:, 0:S5], rot[c][:, S5 : 2 * S5])
                    pprod.append(pp)
                vsum = tpool.tile([P, NV], F16, tag="vsum")
                nc.vector.tensor_add(vsum[:], pprod[0][:], pprod[1][:])
                v = vpool.tile([P, NV], F16, tag="v")
                nc.vector.tensor_add(v[:], vsum[:], pprod[2][:])

                # ---- masked squares on ACT: sum(relu(-v)^2) -> acc ----------
                mrelu = vpool.tile([P, NV], F16, tag="mrelu")
                nc.scalar.activation(mrelu[:], v[:], mybir.ActivationFunctionType.Relu,
                                     scale=-1.0)
                sqj = vpool.tile([P, NV], F16, tag="sqj")
                nc.scalar.activation(sqj[:], mrelu[:],
                                     mybir.ActivationFunctionType.Square,
                                     accum_out=acc[:, rep * G + g : rep * G + g + 1])

                # ---- PE reduction of coplane products over partitions -------
                for i in range(n_chunks):
                    lo = 512 * i
                    hi = min(NCOP, lo + 512)
                    nc.tensor.matmul(psum_cop[:, lo:hi], ones[:], red[:, lo:hi],
                                     start=first, stop=last)

        if loop_cm is not None:
            loop_cm.__exit__(None, None, None)

        # ---- epilogue: PSUM -> SBUF -> DRAM ---------------------------------
        cop_sb = spool.tile([1, NCOP], F32)
        nc.scalar.copy(cop_sb[:], psum_cop[:])
        nc.sync.dma_start(cop_out.ap(), cop_sb[:])
        nc.sync.dma_start(mask_out.ap(), acc[:])

    nc.compile()
    return nc, G


def host_planarize(x: np.ndarray, n_cores: int, K: int) -> np.ndarray:
    """[B,21,3] f32 -> [cores, G, P, 75K] f32: slot layout [c][jj:5][f:5][k]."""
    B = x.shape[0]
    R = B // n_cores
    G = R // (P * K)
    xr = x.reshape(n_cores, G, P, K, 21, 3)
    jidx = (np.arange(5) * 4)[:, None] + np.arange(5)[None, :]  # [f, jj]
    xj = xr[:, :, :, :, jidx, :]                 # [cores,G,P,K,f,jj,3]
    xp = xj.transpose(0, 1, 2, 6, 5, 4, 3)       # [cores,G,P,c,jj,f,K]
    return np.ascontiguousarray(xp).reshape(n_cores, G, P, ROW * K)


_CACHE = {}


def _get_nc(rows_per_core: int, K: int):
    key = (rows_per_core, K)
    if key not in _CACHE:
        _CACHE[key] = build_bass(rows_per_core, K)
    return _CACHE[key]


def kernel(pose23d_pred: np.ndarray) -> np.ndarray:
    x = np.asarray(pose23d_pred, dtype=np.float32)
    assert x.shape == (B_FULL, 21, 3), x.shape
    K = 128
    R = B_FULL // N_CORES
    nc, G = _get_nc(R, K)
    xp = host_planarize(x, N_CORES, K)
    in_maps = [{"x": xp[i]} for i in range(N_CORES)]
    res = run_bass_kernel_spmd(nc, in_maps, list(range(N_CORES)))
    total = 0.0
    for r in res.results:
        total += r["cop_out"].astype(np.float64).sum()
        total += r["mask_out"].astype(np.float64).sum()
    return np.float32(total)


# revision 11
# speedup vs baseline: 824.9293x; 1.0810x over previous
"""JointAngleLoss Trainium2 kernel (8-core data-parallel).

Input : pose23d_pred [524288, 21, 3] float32
Output: scalar float32 loss (matches reference.reference)

Strategy: pure data-parallel over the batch dim; each of 8 NeuronCores handles
65536 rows. Host pre-permutes the input (dtype preserving) into a per-partition
slot layout J[c][jj][f][k] (jj = joint-within-finger, duplicating the 4 shared
joints: 75 floats per row) so that EVERY device-side vector operand is a flat
contiguous fp16 slice - this keeps the DVE in its 2x_1P packed perf mode.
Device pipeline per group:
  DMA fp32 -> ACT cast->fp16 -> DVE bones/crosses/dots (flat fp16 2x)
  -> ACT relu(-v)^2 with fp32 accum_out  +  PE ones-matmul reduces the
  coplanarity products into PSUM fp32 across groups.
Host sums the per-core partials in float64.
"""

import sys

for _p in ("/opt/trn_rl_repo", "/root/.axon_site/_ro/trn_rl_repo"):
    if _p not in sys.path:
        sys.path.append(_p)

import numpy as np

import concourse.bacc as bacc
import concourse.mybir as mybir
from concourse import tile
from concourse.bass_utils import run_bass_kernel_spmd
from contextlib import ExitStack

N_CORES = 8
P = 128          # SBUF partitions
B_FULL = 524288  # total batch
ROW = 75         # 3 comps * 5 joint-slots * 5 fingers (shared joints duplicated)

F16 = mybir.dt.float16
F32 = mybir.dt.float32


def build_bass(rows_per_core: int, K: int, reps: int = 1, hw_loop: int = 1,
               pool_bones: bool = False):
    """rows_per_core = P * K * G.  K = rows per partition slot per group.

    reps>1 unrolls the compute (timing); hw_loop>1 wraps it in a device-side
    For_i (timing; outputs = last iteration's = one correct pass).
    """
    assert rows_per_core % (P * K) == 0
    G = rows_per_core // (P * K)
    FK = ROW * K          # fp32 elems per partition per group (75*K)
    CJ = 25 * K           # joint elems per component (5jj*5f*K)
    CB = 20 * K           # bone elems per component  (4jj*5f*K)
    S5 = 5 * K            # one [f][k] slab
    NCOP = 3 * S5         # coplane products per partition
    NV = 2 * S5           # v values per partition

    nc = bacc.Bacc("TRN2", target_bir_lowering=False, debug=False)

    x = nc.dram_tensor("x", [G, P, FK], F32, kind="ExternalInput")
    cop_out = nc.dram_tensor("cop_out", [1, NCOP], F32, kind="ExternalOutput")
    mask_out = nc.dram_tensor("mask_out", [P, G * reps], F32, kind="ExternalOutput")

    with tile.TileContext(nc) as tc, ExitStack() as ctx:
        xpool = ctx.enter_context(tc.tile_pool(name="xpool", bufs=2))
        hpool = ctx.enter_context(tc.tile_pool(name="hpool", bufs=1))
        bpool = ctx.enter_context(tc.tile_pool(name="bpool", bufs=2))
        rpool = ctx.enter_context(tc.tile_pool(name="rpool", bufs=2))
        tpool = ctx.enter_context(tc.tile_pool(name="tpool", bufs=1))
        vpool = ctx.enter_context(tc.tile_pool(name="vpool", bufs=2))
        spool = ctx.enter_context(tc.tile_pool(name="spool", bufs=1))
        psum = ctx.enter_context(tc.tile_pool(name="psum", bufs=1, space="PSUM"))

        ones = spool.tile([P, 1], F16)
        nc.gpsimd.memset(ones[:], 1.0)
        acc = spool.tile([P, G * reps], F32)
        psum_cop = psum.tile([1, NCOP], F32)

        n_chunks = (NCOP + 511) // 512

        loop_cm = tc.For_i(0, hw_loop, 1) if hw_loop > 1 else None
        if loop_cm is not None:
            loop_cm.__enter__()

        for rep in range(reps):
            for g in range(G):
                first = rep == 0 and g == 0
                last = rep == reps - 1 and g == G - 1

                # ---- load + cast (all flat, split in half for earlier start)
                xt = xpool.tile([P, FK], F32)
                xh = hpool.tile([P, FK], F16)
                half = FK // 2
                for h in range(2):
                    sl = slice(h * half, (h + 1) * half)
                    nc.sync.dma_start(xt[:, sl], x.ap()[g][:, sl])
                    nc.scalar.copy(xh[:, sl], xt[:, sl])

                # ---- bones: B[c][jj][f][k] = J[c][jj+1][f][k]-J[c][jj][f][k]
                bones = bpool.tile([P, 3 * CB], F16)
                beng = nc.gpsimd if pool_bones else nc.vector
                for c in range(3):
                    beng.tensor_sub(
                        bones[:, c * CB : (c + 1) * CB],
                        xh[:, c * CJ + S5 : c * CJ + CJ],
                        xh[:, c * CJ : c * CJ + CB],
                    )

                def bslab(c, jj):  # bone block, flat [P, 5K]
                    o = c * CB + jj * S5
                    return bones[:, o : o + S5]

                # ---- crosses: R_c[qh][f][k], qh: 0=palm 1=mid 2=tip ---------
                # rot[c] = B_{c1}[jj=qh+1]*B_{c2}[jj=qh] - B_{c2}[jj=qh+1]*B_{c1}[jj=qh]
                rot = []
                for c in range(3):
                    c1, c2 = (c + 1) % 3, (c + 2) % 3
                    m1 = tpool.tile([P, NCOP], F16, tag="m1")
                    m2 = tpool.tile([P, NCOP], F16, tag="m2")
                    rc = rpool.tile([P, NCOP], F16, tag=f"rot{c}")
                    nc.vector.tensor_mul(
                        m1[:], bones[:, c1 * CB + S5 : c1 * CB + CB],
                        bones[:, c2 * CB : c2 * CB + NCOP])
                    nc.vector.tensor_mul(
                        m2[:], bones[:, c2 * CB + S5 : c2 * CB + CB],
                        bones[:, c1 * CB : c1 * CB + NCOP])
                    nc.vector.tensor_sub(rc[:], m1[:], m2[:])
                    rot.append(rc)

                # ---- coplane products: (palm_c + mid_c) * b4_c  (all flat) --
                red = vpool.tile([P, NCOP], F16, tag="red")
                for c in range(3):
                    pc = tpool.tile([P, S5], F16, tag="pc")
                    nc.vector.tensor_add(pc[:], rot[c][:, 0:S5], rot[c][:, S5:2 * S5])
                    nc.vector.tensor_mul(
                        red[:, c * S5 : (c + 1) * S5], pc[:], bslab(c, 3))

                # ---- v1 = tip.mid, v2 = palm.mid ----------------------------
                pprod = []
                for c in range(3):
                    pp = tpool.tile([P, NV], F16, tag=f"pp{c}")
                    nc.vector.tensor_mul(
                        pp[:, 0:S5], rot[c][:, 2 * S5 : 3 * S5], rot[c][:, S5 : 2 * S5])
                    nc.vector.tensor_mul(
                        pp[:, S5:NV], rot[c][:, 0:S5], rot[c][:, S5 : 2 * S5])
                    pprod.append(pp)
                vsum = tpool.tile([P, NV], F16, tag="vsum")
                nc.vector.tensor_add(vsum[:], pprod[0][:], pprod[1][:])
                v = vpool.tile([P, NV], F16, tag="v")
                nc.vector.tensor_add(v[:], vsum[:], pprod[2][:])

                # ---- masked squares on ACT: sum(relu(-v)^2) -> acc ----------
                mrelu = vpool.tile([P, NV], F16, tag="mrelu")
                nc.scalar.activation(mrelu[:], v[:], mybir.ActivationFunctionType.Relu,
                                     scale=-1.0)
                sqj = vpool.tile([P, NV], F16, tag="sqj")
                nc.scalar.activation(sqj[:], mrelu[:],
                                     mybir.ActivationFunctionType.Square,
                                     accum_out=acc[:, rep * G + g : rep * G + g + 1])

                # ---- PE reduction of coplane products over partitions -------
                for i in range(n_chunks):
                    lo = 512 * i
                    hi = min(NCOP, lo + 512)
                    nc.tensor.matmul(psum_cop[:, lo:hi], ones[:], red[:, lo:hi],
                                     start=first, stop=last)

        if loop_cm is not None:
            loop_cm.__exit__(None, None, None)

        # ---- epilogue: PSUM -> SBUF -> DRAM ---------------------------------
        cop_sb = spool.tile([1, NCOP], F32)
        nc.scalar.copy(cop_sb[:], psum_cop[:])
        nc.sync.dma_start(cop_out.ap(), cop_sb[:])
        nc.sync.dma_start(mask_out.ap(), acc[:])

    nc.compile()
    return nc, G


def host_planarize(x: np.ndarray, n_cores: int, K: int) -> np.ndarray:
    """[B,21,3] f32 -> [cores, G, P, 75K] f32: slot layout [c][jj:5][f:5][k]."""
    B = x.shape[0]
    R = B // n_cores
    G = R // (P * K)
    xr = x.reshape(n_cores, G, P, K, 21, 3)
    jidx = (np.arange(5) * 4)[:, None] + np.arange(5)[None, :]  # [f, jj]
    xj = xr[:, :, :, :, jidx, :]                 # [cores,G,P,K,f,jj,3]
    xp = xj.transpose(0, 1, 2, 6, 5, 4, 3)       # [cores,G,P,c,jj,f,K]
    return np.ascontiguousarray(xp).reshape(n_cores, G, P, ROW * K)


_CACHE = {}


def _get_nc(rows_per_core: int, K: int):
    key = (rows_per_core, K)
    if key not in _CACHE:
        _CACHE[key] = build_bass(rows_per_core, K)
    return _CACHE[key]


def kernel(pose23d_pred: np.ndarray) -> np.ndarray:
    x = np.asarray(pose23d_pred, dtype=np.float32)
    assert x.shape == (B_FULL, 21, 3), x.shape
    K = 128
    R = B_FULL // N_CORES
    nc, G = _get_nc(R, K)
    xp = host_planarize(x, N_CORES, K)
    in_maps = [{"x": xp[i]} for i in range(N_CORES)]
    res = run_bass_kernel_spmd(nc, in_maps, list(range(N_CORES)))
    total = 0.0
    for r in res.results:
        total += r["cop_out"].astype(np.float64).sum()
        total += r["mask_out"].astype(np.float64).sum()
    return np.float32(total)
